# revision 1
# baseline (speedup 1.0000x reference)
"""Distillation-trainer loss kernel for Trainium2 (8 NeuronCores).

Computes  loss = mean((attn(q,k,v) - attn(q,ck,cv))**2)  for
q:[1,8,1024,128], k/v:[1,8,8192,128], ck/cv:[1,8,1024,128] fp32.

Sharding: one kv-head per core (h axis, 8 heads / 8 cores). Each core
computes its head's squared-error partial sums; the host adds the 8
partials and divides by the element count (the "all-reduce" of the
scalar loss).

Per-core algorithm (head h):
  - load K/Q/CK, transpose to [d, n] layout via PE (fp32 transpose,
    cast to bf16 on the PSUM->SBUF copy).
  - scoresT[n, q] = KT-tile.T @ QT on PE in bf16 (out fp32 PSUM).
  - exp on ACT:  expT = Exp(scoresT * 1/sqrt(d)) -> bf16 SBUF. No
    max-subtraction: scores ~ N(0,1); max over 8k samples < 5, exp
    stays < ~150 which is safely inside fp32/bf16 range.
  - PV: z'[q, 0:128] + S[q] in one accumulation: stationary = expT
    chunk [128n, 128q], moving = v' [128n, 129] where v' has a ones
    column appended; PSUM accumulates over the 64 n-tiles.
  - z = z'[:, :128] * (1 / z'[:, 128]) on DVE; same for compressed;
    (z - zc)^2 row-reduced into per-partition partials.
"""

import numpy as np

import concourse.bass as bass
import concourse.mybir as mybir
import concourse.tile as tile
from concourse import bacc
from concourse.masks import make_identity
from concourse.bass_utils import run_bass_kernel_spmd

F32 = mybir.dt.float32
BF16 = mybir.dt.bfloat16
FP8 = mybir.dt.float8e4     # e4m3: PV operands (exp probs, values)
AF = mybir.ActivationFunctionType
ALU = mybir.AluOpType

B, H, Q, N, NC, D = 1, 8, 1024, 8192, 1024, 128
N_CORES = 8
SCALE = 1.0 / float(np.sqrt(D))

QC = 256                   # q chunk width for the scores moving operand
N_QC = Q // QC             # 4
GT = 6                     # n-tiles per PSUM scores region / ACT call (3 banks)
NT = N // 128              # 64 teacher n-tiles
NCT = NC // 128            # 8 compressed n-tiles


def _emit(nc: bass.Bass, tc: tile.TileContext, qh, kh, vh, ckh, cvh, out_dram):
    ctxs = []

    def pool(**kw):
        p = tc.tile_pool(**kw)
        ctxs.append(p)
        return p.__enter__()

    pconst = pool(name="pconst", bufs=1)
    pstage = pool(name="pstage", bufs=4)
    pex = pool(name="pex", bufs=3)
    psmall = pool(name="psmall", bufs=4)
    psc = pool(name="psc", bufs=2, space="PSUM")
    pz = pool(name="pz", bufs=1, space="PSUM")

    # ---- persistent SBUF tensors ----
    ident = pconst.tile([128, 128], BF16, tag="ident")
    make_identity(nc, ident[:])

    kT = pconst.tile([128, NT, 128], BF16, tag="kT")       # [d, t, n]
    vb = pconst.tile([128, NT, 129], BF16, tag="vb")        # [n, t, d+1]
    qT = pconst.tile([128, Q], BF16, tag="qT")             # [d, q]
    ckT = pconst.tile([128, NCT, 128], BF16, tag="ckT")
    cvb = pconst.tile([128, NCT, 129], BF16, tag="cvb")
    zcomp = pconst.tile([128, Q // 128, 128], F32, tag="zcomp")  # [q, qt, d]
    accq = pconst.tile([128, Q // 128], F32, tag="accq")

    nc.gpsimd.memset(vb[:, :, 128:129], 1.0)
    nc.gpsimd.memset(cvb[:, :, 128:129], 1.0)

    # Warm the ACT exp table while prep DMAs run, so the ~2.7us
    # ACT_TABLE_LOAD is off the first real exp's critical path.
    warm = psmall.tile([128, 1], F32, tag="warm")
    nc.gpsimd.memset(warm[:], 0.0)
    warm2 = psmall.tile([128, 1], F32, tag="warm2")
    nc.scalar.activation(warm2[:], warm[:], AF.Exp)

    # ---- load + transpose K, load V (cast fp32 -> bf16) ----
    def load_transposed_chunk(src, dst, g, tag):
        # 512 rows of src -> dst[:, 4g:4g+4, :] in [d, t, n] layout:
        # DMA load, DVE cast to bf16, PE-transpose each 128x128 tile.
        # Transpose PSUM scratch borrows the scores pool's slots (tag
        # "sp") so prep+main stay within the 8 PSUM banks.
        stg = pstage.tile([128, 4, 128], F32, tag=tag)
        ap = src[g * 512:(g + 1) * 512, :].rearrange("(i p) d -> p i d", p=128)
        nc.sync.dma_start(out=stg[:], in_=ap)
        kb = pstage.tile([128, 4, 128], BF16, tag=tag + "b")
        nc.vector.tensor_copy(kb[:], stg[:])
        tp = psc.tile([128, 4, 128], BF16, tag="sp")
        for j in range(4):
            nc.tensor.transpose(tp[:, j, :], kb[:, j, :], ident[:])
        nc.vector.tensor_copy(dst[:, 4 * g:4 * g + 4, :], tp[:])

    def load_values_chunk(src, dst, g, tag):
        # 512 rows of src -> dst[:, 4g:4g+4, 0:128] bf16 ([n, t, d])
        stg = pstage.tile([128, 4, 128], F32, tag=tag)
        ap = src[g * 512:(g + 1) * 512, :].rearrange("(i p) d -> p i d", p=128)
        nc.sync.dma_start(out=stg[:], in_=ap)
        nc.vector.tensor_copy(dst[:, 4 * g:4 * g + 4, 0:128], stg[:])

    def load_transposed(src, dst, n_tiles, tag):
        for g in range(n_tiles // 4):
            load_transposed_chunk(src, dst, g, tag)

    def load_values(src, dst, n_tiles, tag):
        for g in range(n_tiles // 4):
            load_values_chunk(src, dst, g, tag)

    # q: [1024, 128] -> qT [128, 1024]
    stq = pstage.tile([128, 8, 128], F32, tag="stq")
    nc.sync.dma_start(out=stq[:], in_=qh[:, :].rearrange("(i p) d -> p i d", p=128))
    qb = pstage.tile([128, 8, 128], BF16, tag="stqb")
    nc.vector.tensor_copy(qb[:], stq[:])
    for gg in range(2):
        tp = psc.tile([128, 4, 128], BF16, tag="sp")
        for j in range(4):
            nc.tensor.transpose(tp[:, j, :], qb[:, 4 * gg + j, :], ident[:])
        nc.vector.tensor_copy(
            qT[:, 512 * gg:512 * (gg + 1)].rearrange("p (a b) -> p a b", a=4),
            tp[:])

    # Small compressed-side operands first: the compressed attention
    # phase needs only ck/cv/q (1.5 MB), and its compute hides the
    # 8 MB k/v stream, whose chunks are interleaved into the
    # compressed phase below.
    load_transposed(ckh, ckT, NCT, "stk")
    load_values(cvh, cvb, NCT, "stv")

    # ---- attention + softmax-PV for one q-chunk of 256 ----
    def attend(keysT, vals, n_tiles, qc):
        """Returns (za, zb) PSUM tiles [128, 129] = [z' | S] per q-half.
        Two separate tiles: PSUM accumulation-group tracking is bank-
        granular, so the two interleaved groups need distinct banks."""
        za = pz.tile([128, 129], F32, tag="za")
        zb = pz.tile([128, 129], F32, tag="zb")
        qs = qT[:, qc * QC:(qc + 1) * QC]

        def pv_ops(ex, t0, gn):
            ops = []
            for j in range(gn):
                t = t0 + j
                st = dict(start=(t == 0), stop=(t == n_tiles - 1))
                for c0, zp in ((0, za), (128, zb)):
                    ops.append(lambda j=j, c0=c0, zp=zp, st=st, t=t:
                               nc.tensor.matmul(zp[:], ex[:, j, c0:c0 + 128],
                                                vals[:, t, :], **st))
            return ops

        def emit_pv(ex, t0, gn):
            for op in pv_ops(ex, t0, gn):
                op()

        # Ramp group sizes: small first exp groups shrink the pipeline-
        # fill bubble (PE waits on the first ACT of each attend).
        sizes = []
        left = n_tiles
        for want in (2, 4):
            if left > GT:
                sizes.append(want)
                left -= want
        while left > 0:
            gn = min(GT, left)
            sizes.append(gn)
            left -= gn

        pending = None
        t0 = 0
        for gn in sizes:
            sp = psc.tile([128, GT, QC], F32, tag="sp")
            for j in range(gn):
                nc.tensor.matmul(sp[:, j, :], keysT[:, t0 + j, :], qs,
                                 start=True, stop=True)
            if pending is not None:
                emit_pv(*pending)
            ex = pex.tile([128, GT, QC], BF16, tag="ex")
            nc.scalar.activation(ex[:, 0:gn, :], sp[:, 0:gn, :], AF.Exp,
                                 scale=SCALE)
            pending = (ex, t0, gn)
            t0 += gn
        emit_pv(*pending)
        return za, zb

    # Phase 1: compressed attention for all q chunks; normalized zc
    # lands in SBUF (zcomp). The heavy k/v loads are interleaved per
    # qc so their DMA streams behind this phase's compute.
    for qc in range(N_QC):
        za, zb = attend(ckT, cvb, NCT, qc)
        for g in range(4 * qc, 4 * qc + 4):
            load_transposed_chunk(kh, kT, g, "stk")
        for h, zp in ((0, za), (1, zb)):
            qt = qc * 2 + h
            zr = psmall.tile([128, 129], F32, tag="zr")
            nc.vector.tensor_copy(zr[:], zp[:])
            inv = psmall.tile([128, 1], F32, tag="inv")
            nc.vector.reciprocal(inv[:], zr[:, 128:129])
            nc.vector.tensor_scalar_mul(zcomp[:, qt, :], zr[:, 0:128], inv[:])
        for g in range(4 * qc, 4 * qc + 4):
            load_values_chunk(vh, vb, g, "stv")

    # Phase 2: teacher attention + MSE partials against stored zc.
    for qc in range(N_QC):
        za, zb = attend(kT, vb, NT, qc)
        for h, zp in ((0, za), (1, zb)):
            qt = qc * 2 + h
            zr = psmall.tile([128, 129], F32, tag="zcr")
            nc.vector.tensor_copy(zr[:], zp[:])
            inv = psmall.tile([128, 1], F32, tag="inv")
            nc.vector.reciprocal(inv[:], zr[:, 128:129])
            zcn = psmall.tile([128, 128], F32, tag="zcn")
            nc.vector.tensor_scalar_mul(zcn[:], zr[:, 0:128], inv[:])
            d = psmall.tile([128, 128], F32, tag="d")
            nc.vector.tensor_sub(d[:], zcn[:], zcomp[:, qt, :])
            d2 = psmall.tile([128, 128], F32, tag="d2")
            nc.vector.tensor_mul(d2[:], d[:], d[:])
            nc.vector.reduce_sum(out=accq[:, qt:qt + 1], in_=d2[:],
                                 axis=mybir.AxisListType.X)

    nc.sync.dma_start(out=out_dram[:], in_=accq[:])

    for p in reversed(ctxs):
        p.__exit__(None, None, None)


_NC_CACHE = None


def build_nc():
    global _NC_CACHE
    if _NC_CACHE is not None:
        return _NC_CACHE
    nc = bacc.Bacc()
    qh = nc.declare_dram_parameter("queries", [Q, D], F32, isOutput=False)
    kh = nc.declare_dram_parameter("keys", [N, D], F32, isOutput=False)
    vh = nc.declare_dram_parameter("values", [N, D], F32, isOutput=False)
    ckh = nc.declare_dram_parameter("c_keys", [NC, D], F32, isOutput=False)
    cvh = nc.declare_dram_parameter("c_values", [NC, D], F32, isOutput=False)
    out = nc.declare_dram_parameter("loss_sums", [128, Q // 128], F32, isOutput=True)
    with tile.TileContext(nc) as tc:
        _emit(nc, tc, qh, kh, vh, ckh, cvh, out)
    nc.compile()
    _NC_CACHE = nc
    return nc


def make_in_maps(queries, keys, values, c_keys, c_values):
    in_maps = []
    for h in range(N_CORES):
        in_maps.append({
            "queries": np.ascontiguousarray(queries[0, h], dtype=np.float32),
            "keys": np.ascontiguousarray(keys[0, h], dtype=np.float32),
            "values": np.ascontiguousarray(values[0, h], dtype=np.float32),
            "c_keys": np.ascontiguousarray(c_keys[0, h], dtype=np.float32),
            "c_values": np.ascontiguousarray(c_values[0, h], dtype=np.float32),
        })
    return in_maps


def run_cores(in_maps, trace=False, **kw):
    nc = build_nc()
    return run_bass_kernel_spmd(nc, in_maps, list(range(N_CORES)),
                                trace=trace, **kw)


def kernel(queries, keys, values, c_keys, c_values):
    res = run_cores(make_in_maps(queries, keys, values, c_keys, c_values))
    total = sum(float(r["loss_sums"].astype(np.float64).sum())
                for r in res.results)
    loss = total / float(B * H * Q * D)
    return np.asarray(loss, dtype=np.float32)



# revision 5
# speedup vs baseline: 1.0006x; 1.0006x over previous
"""Distillation-trainer loss kernel for Trainium2 (8 NeuronCores).

Computes  loss = mean((attn(q,k,v) - attn(q,ck,cv))**2)  for
q:[1,8,1024,128], k/v:[1,8,8192,128], ck/cv:[1,8,1024,128] fp32.

Sharding: one kv-head per core (h axis, 8 heads / 8 cores). The host
adds the 8 per-core partial sums and divides by the element count.

Host-side prep (not on the device critical path): per head, Q/K/CK are
transposed to [d, n] and cast to bf16; V/CV are cast to fp8e4m3 and
pre-swizzled to the SBUF tile layout [n%128, n//128, d] so every DMA is
a contiguous row copy. This removes all PE transposes and all DVE
dtype-cast traffic from the device.

Per-core algorithm (head h), scores kept in [n, q] orientation:
  - scoresT[nt, q] = kT-tile.T @ qT on PE, bf16, fp32 PSUM, 2 MMs of
    free-dim 512 (one PSUM bank each).
  - expT = Exp(scoresT * 1/sqrt(d) - 3) -> fp8e4m3 SBUF on ACT. The -3
    shift cancels in softmax normalization and keeps exp <= e^2.5=12.2,
    far below the TRN fp8e4 max of 240 (scores ~ N(0,1), max < 5.5).
  - PV: DoubleRow fp8 matmuls over n-tile pairs: stationary
    v[128, 2, 128], moving expT[128, 2, 512], accumulating
    zT[d, q] over all pairs (2x PE throughput).
  - softmax denominator: sacc[nlane, q] += expT tile on DVE (fp16);
    after the loop an all-ones [128,128] matmul reduces over partitions
    and broadcasts S[q] to all 128 partitions in one shot; DVE
    reciprocal gives invS.
  - zc = zcT * invSc stored to SBUF (compressed pass first); teacher
    pass computes d = zT*invS - zc, then (d*d) row-reduced into
    [128, 1] partials.
"""

import numpy as np

import concourse.bass as bass
import concourse.mybir as mybir
import concourse.tile as tile
from concourse import bacc
from concourse.bass_utils import run_bass_kernel_spmd

F32 = mybir.dt.float32
F16 = mybir.dt.float16
BF16 = mybir.dt.bfloat16
FP8 = mybir.dt.float8e4
AF = mybir.ActivationFunctionType
ALU = mybir.AluOpType
DR = mybir.MatmulPerfMode.DoubleRow

B, H, Q, N, NC, D = 1, 8, 1024, 8192, 1024, 128
N_CORES = 8
SCALE = 1.0 / float(np.sqrt(D))
EXP_BIAS = -3.0            # cancels in softmax; keeps exp in fp8e4 range

NT = N // 128              # 64 teacher n-tiles
NCT = NC // 128            # 8 compressed n-tiles
KCH = 8                    # kT/v DMA chunks (n-tiles per chunk)


def _emit(nc: bass.Bass, tc: tile.TileContext, qT_d, kT_d, ckT_d, v_d, cv_d,
          out_d):
    ctxs = []

    def pool(**kw):
        p = tc.tile_pool(**kw)
        ctxs.append(p)
        return p.__enter__()

    pconst = pool(name="pconst", bufs=1)
    psacc = pool(name="psacc", bufs=2)
    pinv = pool(name="pinv", bufs=2)
    pex = pool(name="pex", bufs=3)
    psmall = pool(name="psmall", bufs=2)
    psc = pool(name="psc", bufs=3, space="PSUM")   # 3 x 2 banks
    pz = pool(name="pz", bufs=1, space="PSUM")     # 1 x 2 banks

    # ---- persistent SBUF tensors ----
    qT = pconst.tile([128, Q], BF16, tag="qT")          # [d, q]
    kT = pconst.tile([128, N], BF16, tag="kT")          # [d, n]
    ckT = pconst.tile([128, NC], BF16, tag="ckT")       # [d, n]
    v = pconst.tile([128, NT, 128], FP8, tag="v")       # [nlane, t, d]
    cv = pconst.tile([128, NCT, 128], FP8, tag="cv")
    ones = pconst.tile([128, 128], F16, tag="ones")
    zcomp = pconst.tile([128, Q], F32, tag="zcomp")     # [d, q] normalized zc

    nc.gpsimd.memset(ones[:], 1.0)
    bias_ap = pconst.tile([128, 1], F32, tag="bias")
    nc.gpsimd.memset(bias_ap[:], EXP_BIAS)

    # Warm the ACT exp table while the first DMAs run (~2.7us otherwise
    # on the first real exp's critical path).
    warm = psmall.tile([128, 1], F32, tag="warm")
    nc.gpsimd.memset(warm[:], 0.0)
    warm2 = psmall.tile([128, 1], F32, tag="warm2")
    nc.scalar.activation(warm2[:], warm[:], AF.Exp)

    # ---- input DMAs (compressed-pass operands first) ----
    nc.sync.dma_start(out=qT[:], in_=qT_d[:, :])
    nc.sync.dma_start(out=ckT[:], in_=ckT_d[:, :])
    nc.sync.dma_start(out=cv[:], in_=cv_d[:, :].rearrange("p (t d) -> p t d", d=128))
    for c in range(NT // KCH):
        sl = slice(c * KCH * 128, (c + 1) * KCH * 128)
        nc.sync.dma_start(out=kT[:, sl], in_=kT_d[:, sl])
    for c in range(NT // KCH):
        nc.sync.dma_start(
            out=v[:, c * KCH:(c + 1) * KCH, :],
            in_=v_d[:, c * KCH * 128:(c + 1) * KCH * 128].rearrange(
                "p (t d) -> p t d", d=128))

    def attention(keysT, vals, nt):
        """One softmax-attention pass. Returns (z_psum [d,q] unnormalized,
        invS SBUF [128, q] broadcast reciprocal denominator)."""
        sacc = psacc.tile([128, Q], F16, tag="sacc")
        nc.gpsimd.memset(sacc[:], 0.0)
        zp = pz.tile([128, Q], F32, tag="z")
        npairs = nt // 2
        for tp in range(npairs):
            ex = pex.tile([128, 2, Q], FP8, tag="ex")
            for j in (0, 1):
                t = 2 * tp + j
                sc = psc.tile([128, Q], F32, tag="sc")
                for h in (0, 1):
                    nc.tensor.matmul(sc[:, 512 * h:512 * (h + 1)],
                                     keysT[:, 128 * t:128 * (t + 1)],
                                     qT[:, 512 * h:512 * (h + 1)],
                                     start=True, stop=True)
                nc.scalar.activation(ex[:, j, :], sc[:], AF.Exp,
                                     bias=bias_ap[:], scale=SCALE)
                nc.vector.tensor_tensor(sacc[:], sacc[:], ex[:, j, :], ALU.add)
            st = dict(start=(tp == 0), stop=(tp == npairs - 1))
            for h in (0, 1):
                nc.tensor.matmul(zp[:, 512 * h:512 * (h + 1)],
                                 vals[:, 2 * tp:2 * tp + 2, :],
                                 ex[:, :, 512 * h:512 * (h + 1)],
                                 perf_mode=DR, **st)
        # S[q] = sum over partitions of sacc, broadcast to 128 partitions
        sb = psc.tile([128, Q], F32, tag="sc")
        for h in (0, 1):
            nc.tensor.matmul(sb[:, 512 * h:512 * (h + 1)], ones[:],
                             sacc[:, 512 * h:512 * (h + 1)],
                             start=True, stop=True)
        inv = pinv.tile([128, Q], F32, tag="inv")
        nc.vector.reciprocal(inv[:], sb[:])
        return zp, inv

    # Phase 1: compressed attention -> zcomp (normalized, SBUF)
    zcp, invc = attention(ckT, cv, NCT)
    nc.vector.tensor_tensor(zcomp[:], zcp[:], invc[:], ALU.mult)

    # Phase 2: teacher attention + MSE partials
    zp, inv = attention(kT, v, NT)
    zn = psacc.tile([128, Q], F32, tag="zn")
    nc.vector.tensor_tensor(zn[:], zp[:], inv[:], ALU.mult)
    dd = psacc.tile([128, Q], F32, tag="dd")
    nc.vector.tensor_tensor(dd[:], zn[:], zcomp[:], ALU.subtract)
    scr = psacc.tile([128, Q], F32, tag="scr")
    nc.vector.tensor_tensor(scr[:], dd[:], dd[:], ALU.mult)
    acc = psmall.tile([128, 1], F32, tag="acc")
    nc.vector.reduce_sum(out=acc[:], in_=scr[:], axis=mybir.AxisListType.X)
    nc.sync.dma_start(out=out_d[:], in_=acc[:])

    for p in reversed(ctxs):
        p.__exit__(None, None, None)


_NC_CACHE = None


def build_nc():
    global _NC_CACHE
    if _NC_CACHE is not None:
        return _NC_CACHE
    nc = bacc.Bacc()
    qT_d = nc.declare_dram_parameter("qT", [128, Q], BF16, isOutput=False)
    kT_d = nc.declare_dram_parameter("kT", [128, N], BF16, isOutput=False)
    ckT_d = nc.declare_dram_parameter("ckT", [128, NC], BF16, isOutput=False)
    v_d = nc.declare_dram_parameter("v", [128, N], FP8, isOutput=False)
    cv_d = nc.declare_dram_parameter("cv", [128, NC], FP8, isOutput=False)
    out_d = nc.declare_dram_parameter("loss_sums", [128, 1], F32, isOutput=True)
    with tile.TileContext(nc) as tc:
        _emit(nc, tc, qT_d, kT_d, ckT_d, v_d, cv_d, out_d)
    nc.compile()
    _NC_CACHE = nc
    return nc


_BF16_NP = mybir.dt.np(BF16)
_FP8_NP = mybir.dt.np(FP8)


def _swizzle_v(x):
    # [n, d] f32 -> [128, n] fp8 in SBUF layout [nlane, ntile, d]
    n, d = x.shape
    t = n // 128
    xw = x.reshape(t, 128, d).transpose(1, 0, 2).reshape(128, n)
    return np.ascontiguousarray(xw.astype(_FP8_NP))


def make_in_maps(queries, keys, values, c_keys, c_values):
    queries = np.asarray(queries, dtype=np.float32)
    keys = np.asarray(keys, dtype=np.float32)
    values = np.asarray(values, dtype=np.float32)
    c_keys = np.asarray(c_keys, dtype=np.float32)
    c_values = np.asarray(c_values, dtype=np.float32)
    in_maps = []
    for h in range(N_CORES):
        in_maps.append({
            "qT": np.ascontiguousarray(queries[0, h].T.astype(_BF16_NP)),
            "kT": np.ascontiguousarray(keys[0, h].T.astype(_BF16_NP)),
            "ckT": np.ascontiguousarray(c_keys[0, h].T.astype(_BF16_NP)),
            "v": _swizzle_v(values[0, h]),
            "cv": _swizzle_v(c_values[0, h]),
        })
    return in_maps


def run_cores(in_maps, trace=False, **kw):
    nc = build_nc()
    return run_bass_kernel_spmd(nc, in_maps, list(range(N_CORES)),
                                trace=trace, **kw)


def kernel(queries, keys, values, c_keys, c_values):
    res = run_cores(make_in_maps(queries, keys, values, c_keys, c_values))
    total = sum(float(r["loss_sums"].astype(np.float64).sum())
                for r in res.results)
    loss = total / float(B * H * Q * D)
    return np.asarray(loss, dtype=np.float32)
